# revision 30
# baseline (speedup 1.0000x reference)
"""GroupedQueryAttention Trainium2 kernel (8 NeuronCores, SPMD).

Sharding: core c -> (batch b = c // 4, kv-group g = c % 4).
Each core computes q/k/v projections for its 4 query heads + 1 kv head,
partial-RoPE, causal attention; the per-group attention outputs o are
exchanged with a per-tile AllToAll (each core ends up with the full
2048 o-features for its 128-token slice of the tile), then every core
applies the full out-projection locally -- no post-projection reduction
is needed and only the small pre-projection o travels on the links.

Pipeline (single TileContext, per q-tile j): project(j) -> attend(j)
(AllToAll fires per head-pair as soon as its o rows are normalized) ->
out-project(j-1).  The list scheduler fills PE gaps during the
ACT-bound attention stretches with projection / out-projection matmuls.

All device matmuls run in bf16 (fp32 PSUM accumulation). The host
pre-transposes the operands so the contraction dim lands on SBUF
partitions everywhere with no on-device transposes:
  xt   = x[b].T                  [D, S]
  wqt  = perm(wq)[group].T       [D, 512]   (rows RoPE-deinterleaved)
  wkt  = perm(wk)[group].T       [D, 128]
  wvt  = wv[group].T             [D, 128]
  wot  = wo.T                    [D, D]     (full; o features are global)
The RoPE deinterleave permutation reorders each head's first 64 dims to
[evens, odds]; since q and k use the same permutation, q.k dot products
are unchanged and it never needs undoing.

Causal structure: for the diagonal 128-row k-chunks only the q columns
at-or-right-of the chunk are computed (scores, exp, p@V and the
denominator matmul are all narrowed); the 128x128 block on the fine
diagonal is masked with a single lower-triangular mask.
"""

import math
import sys

sys.path.insert(0, "/opt/trn_rl_repo")

import numpy as np  # noqa: E402

D_MODEL = 2048
N_HEADS = 16
N_KV = 4
HEAD_DIM = 128
ROPE = 64
THETA = 10000.0
B = 2
HG = N_HEADS // N_KV  # 4 query heads per kv group
GD = HG * HEAD_DIM  # 512 o-features per group
N_CORES = 8
GROUPS_A2A = [[0, 1, 2, 3, 4, 5, 6, 7]]

_BUILD_CACHE: dict = {}


def build_kernel(S: int):
    """Build the per-core Bass program for sequence length S (multiple of 512)."""
    import concourse.bass as bass
    import concourse.mybir as mybir
    import concourse.tile as tile
    from concourse import bacc

    assert S % 512 == 0
    P = 128
    QT = 512  # q tile width
    NJ = S // QT  # q tiles
    NO = D_MODEL // P  # contraction chunks for projections (16)
    NS = S // P  # seq chunks of 128
    N2 = D_MODEL // QT  # out-proj column blocks (4)
    bf16 = mybir.dt.bfloat16
    f32 = mybir.dt.float32
    scale = 1.0 / math.sqrt(HEAD_DIM)

    nc = bacc.Bacc(None, target_bir_lowering=False, debug=False, num_devices=N_CORES)

    xt_d = nc.declare_dram_parameter("xt", [D_MODEL, S], bf16, isOutput=False)
    wqt_d = nc.declare_dram_parameter("wqt", [D_MODEL, GD], bf16, isOutput=False)
    wkt_d = nc.declare_dram_parameter("wkt", [D_MODEL, HEAD_DIM], bf16, isOutput=False)
    wvt_d = nc.declare_dram_parameter("wvt", [D_MODEL, HEAD_DIM], bf16, isOutput=False)
    wot_d = nc.declare_dram_parameter("wot", [D_MODEL, D_MODEL], bf16, isOutput=False)
    cos_d = nc.declare_dram_parameter("cos", [ROPE, S], bf16, isOutput=False)
    sin_d = nc.declare_dram_parameter("sin", [ROPE, S], bf16, isOutput=False)
    msk_d = nc.declare_dram_parameter("mask", [P, P], bf16, isOutput=False)
    idn_d = nc.declare_dram_parameter("ident", [P, P], bf16, isOutput=False)
    out_d = nc.declare_dram_parameter("out", [S // 4, D_MODEL], f32, isOutput=True)

    with tile.TileContext(nc) as tc:
        with (
            tc.tile_pool(name="persist", bufs=1) as persist,
            tc.tile_pool(name="dram", bufs=1, space="DRAM") as dram,
        ):
            # ---- persistent SBUF state ----
            k_sb = persist.tile([P, S], bf16)  # kT (rope'd)
            v_sb = persist.tile([P, NS, HEAD_DIM], bf16)  # v natural per chunk
            cos_sb = persist.tile([ROPE, S], bf16)
            sin_sb = persist.tile([ROPE, S], bf16)
            msk_sb = persist.tile([P, P], bf16)
            idn_sb = persist.tile([P, P], bf16)
            ones_sb = persist.tile([P, P], bf16)
            wkt_sb = persist.tile([P, NO, HEAD_DIM], bf16)
            wvt_sb = persist.tile([P, NO, HEAD_DIM], bf16)
            wqt_sb = persist.tile([P, NO, GD], bf16)
            wot_sb = persist.tile([P, NO, D_MODEL], bf16)

            # 8-core AllToAll: 8 shards of [2 heads x 128 dv, 128 tok]; the
            # cross-batch shards carry unread garbage (mesh needs >4 ranks,
            # so the 4-core exchange rides an 8-core op; each core touches
            # only its batch's 1024-row half via a dynamic offset).
            a2a_in = [
                [
                    dram.tile([8 * 2 * P, P], bf16, name=f"a2ai{j}_{h2}")
                    for h2 in range(2)
                ]
                for j in range(NJ)
            ]
            a2a_out = [
                [
                    dram.tile([8 * 2 * P, P], bf16, name=f"a2ao{j}_{h2}")
                    for h2 in range(2)
                ]
                for j in range(NJ)
            ]
            # last tile exchanges per single head (smaller, earlier ops)
            a2a_in_l = [
                dram.tile([8 * P, P], bf16, name=f"a2ail{h}") for h in range(HG)
            ]
            a2a_out_l = [
                dram.tile([8 * P, P], bf16, name=f"a2aol{h}") for h in range(HG)
            ]
            warm_in = dram.tile([8, P], bf16, name="warm_in")
            warm_out = dram.tile([8, P], bf16, name="warm_out")
            b_half = nc.sync.partition_id() // 4  # 0 or 1: my batch group

            # Bulk loads are split between the two HWDGE engines (SP + ACT)
            # and chunked/ordered so the first projection matmuls start as
            # early as possible: the k-projection needs wkt chunk c and xt
            # chunk o in order, everything else can trail.
            nc.vector.memset(ones_sb[:], 1.0)
            wkt_r = wkt_d.rearrange("(o p) m -> p o m", p=P)
            wvt_r = wvt_d.rearrange("(o p) m -> p o m", p=P)
            wqt_r = wqt_d.rearrange("(o p) m -> p o m", p=P)
            wot_r = wot_d.rearrange("(o p) m -> p o m", p=P)
            xt_r = xt_d.rearrange("(o p) s -> p o s", p=P)

            # tiny dummy collective: pays the Mesh first-op setup cost during
            # the weight preload instead of on the first real exchange
            nc.gpsimd.collective_compute(
                "AllToAll",
                mybir.AluOpType.bypass,
                replica_groups=GROUPS_A2A,
                ins=[warm_in.opt()],
                outs=[warm_out.opt()],
            )

            with (
                tc.tile_pool(name="xt_pool", bufs=2) as xt_pool,
                tc.tile_pool(name="q_pool", bufs=2) as q_pool,
                tc.tile_pool(name="o_pool", bufs=2) as o_pool,
                tc.tile_pool(name="vt_pool", bufs=2) as vt_pool,
                tc.tile_pool(name="pt_pool", bufs=4) as pt_pool,
                tc.tile_pool(name="rcp_pool", bufs=2) as rcp_pool,
                tc.tile_pool(name="orecv_pool", bufs=4) as orecv_pool,
                tc.tile_pool(name="ostg_pool", bufs=8) as ostg_pool,
                tc.tile_pool(name="rtmp", bufs=2) as rtmp,
                tc.tile_pool(name="ps_sc", bufs=2, space="PSUM") as ps_sc,
                tc.tile_pool(name="ps_mm", bufs=2, space="PSUM") as ps_mm,
                tc.tile_pool(name="ps_po", bufs=2, space="PSUM") as ps_po,
                tc.tile_pool(name="ps_den", bufs=2, space="PSUM") as ps_den,
            ):

                def load_xt(j):
                    t = xt_pool.tile([P, NO, QT], bf16, tag="xt")
                    for o2 in range(NO // 2):
                        nc.scalar.dma_start(
                            t[:, 2 * o2 : 2 * o2 + 2, :],
                            xt_r[:, 2 * o2 : 2 * o2 + 2, bass.ts(j, QT)],
                        )
                    return t

                def rope(dst, sl, csl):
                    # rotate-half form on deinterleaved rows:
                    #   rows 0:32 = a (even dims), 32:64 = b (odd dims)
                    #   new[0:64] = old[0:64]*cos64 + swap(old[0:64])*sin64
                    # with cos64 = [cosT; cosT], sin64 = [-sinT; sinT].
                    xs = rtmp.tile([64, QT], bf16, tag="xs")
                    nc.vector.tensor_copy(xs[0:32, :], dst[32:64, sl])
                    nc.vector.tensor_copy(xs[32:64, :], dst[0:32, sl])
                    t = rtmp.tile([64, QT], bf16, tag="t")
                    u = rtmp.tile([64, QT], bf16, tag="u")
                    nc.vector.tensor_mul(t[:], xs[:], sin_sb[:, csl])
                    nc.vector.tensor_mul(u[:], dst[0:64, sl], cos_sb[:, csl])
                    nc.vector.tensor_add(dst[0:64, sl], u[:], t[:])

                def oproj(j, pieces):
                    # pieces: [(recv_tile, [global chunk ids])]
                    # out rows [128j, 128j+128) = tokens 512j+128g..  (host maps)
                    flat = [
                        (t, i, c)
                        for (t, cids) in pieces
                        for i, c in enumerate(cids)
                    ]
                    for n2 in range(N2):
                        ps = ps_mm.tile([P, QT], f32, tag="mm")
                        for k, (t, i, c) in enumerate(flat):
                            nc.tensor.matmul(
                                ps[:],
                                t[:, i, :],
                                wot_sb[:, c, bass.ts(n2, QT)],
                                start=(k == 0),
                                stop=(k == len(flat) - 1),
                            )
                        ostg = ostg_pool.tile([P, QT], f32, tag="ostg")
                        nc.vector.tensor_copy(ostg[:], ps[:])
                        nc.sync.dma_start(
                            out_d[bass.ts(j, P), bass.ts(n2, QT)], ostg[:]
                        )

                # ---- startup loads, interleaved across both HWDGE engines so
                # the k-projection's chunk-o inputs land in consumption order
                xts = [None] * NJ
                xts[0] = xt_pool.tile([P, NO, QT], bf16, name="xt0", tag="xt")
                for o in range(NO):
                    if o % 4 == 0:
                        nc.scalar.dma_start(
                            wkt_sb[:, o : o + 4, :], wkt_r[:, o : o + 4, :]
                        )
                    eng = nc.scalar if o % 2 == 0 else nc.sync
                    eng.dma_start(xts[0][:, o, :], xt_r[:, o, bass.ts(0, QT)])
                for c in range(4):
                    nc.scalar.dma_start(
                        wvt_sb[:, 4 * c : 4 * c + 4, :],
                        wvt_r[:, 4 * c : 4 * c + 4, :],
                    )
                for o2 in range(NO // 2):
                    nc.sync.dma_start(
                        wqt_sb[:, 2 * o2 : 2 * o2 + 2, :],
                        wqt_r[:, 2 * o2 : 2 * o2 + 2, :],
                    )
                nc.scalar.dma_start(cos_sb[:], cos_d[:])
                nc.scalar.dma_start(sin_sb[:], sin_d[:])
                nc.scalar.dma_start(idn_sb[:], idn_d[:])
                nc.scalar.dma_start(msk_sb[:], msk_d[:])
                for o in range(NO):
                    nc.sync.dma_start(wot_sb[:, o, :], wot_r[:, o, :])
                prev_pieces = None

                for j in range(NJ):
                    jsl = bass.ts(j, QT)
                    if j + 1 < NJ:
                        xts[j + 1] = load_xt(j + 1)
                    xt_sb = xts[j]

                    # ---- projections (+RoPE) for tile j ----
                    ps = ps_mm.tile([P, QT], f32, tag="mm")
                    for o in range(NO):
                        nc.tensor.matmul(
                            ps[:], wkt_sb[:, o, :], xt_sb[:, o, :],
                            start=(o == 0), stop=(o == NO - 1),
                        )
                    nc.vector.tensor_copy(k_sb[:, jsl], ps[:])
                    rope(k_sb, jsl, jsl)

                    ps = ps_mm.tile([P, QT], f32, tag="mm")
                    for o in range(NO):
                        nc.tensor.matmul(
                            ps[:], wvt_sb[:, o, :], xt_sb[:, o, :],
                            start=(o == 0), stop=(o == NO - 1),
                        )
                    vt_sb = vt_pool.tile([P, QT], bf16, tag="vt")
                    nc.vector.tensor_copy(vt_sb[:], ps[:])
                    for i in range(QT // P):
                        # PE transpose-mode: [dv, tok] -> [tok, dv], no DMA
                        tps = ps_mm.tile([P, P], bf16, tag="mm")
                        nc.tensor.transpose(
                            tps[:], vt_sb[:, bass.ts(i, P)], idn_sb[:]
                        )
                        nc.vector.tensor_copy(v_sb[:, 4 * j + i, :], tps[:])

                    q_sb = q_pool.tile([P, HG, QT], bf16, tag="q")
                    for h in range(HG):
                        ps = ps_mm.tile([P, QT], f32, tag="mm")
                        for o in range(NO):
                            nc.tensor.matmul(
                                ps[:], wqt_sb[:, o, bass.ts(h, P)], xt_sb[:, o, :],
                                start=(o == 0), stop=(o == NO - 1),
                            )
                        nc.vector.tensor_copy(q_sb[:, h, :], ps[:])
                        rope(q_sb[:, h, :], slice(0, QT), jsl)

                    # ---- attention for tile j ----
                    o_sb = o_pool.tile([P, HG, QT], bf16, tag="o")
                    for h in range(HG):
                        po = ps_po.tile([P, QT], f32, tag="po")
                        den = ps_den.tile([P, QT], f32, tag="den")
                        nk = 4 * (j + 1)
                        for c in range(nk):
                            r = c - 4 * j  # >=0 on the diagonal block
                            off = P * r if r >= 0 else 0
                            w = QT - off
                            sc = ps_sc.tile([P, QT], f32, tag="sc")
                            nc.tensor.matmul(
                                sc[:, 0:w],
                                k_sb[:, bass.ts(c, P)],
                                q_sb[:, h, off:QT],
                                start=True, stop=True,
                            )
                            pt = pt_pool.tile([P, QT], bf16, tag="pt")
                            nc.scalar.activation(
                                pt[:, 0:w], sc[:, 0:w],
                                mybir.ActivationFunctionType.Exp,
                                scale=scale,
                            )
                            if r >= 0:
                                nc.vector.tensor_mul(
                                    pt[:, 0:P], pt[:, 0:P], msk_sb[:]
                                )
                            nc.tensor.matmul(
                                po[:, off:QT], v_sb[:, c, :], pt[:, 0:w],
                                start=(c == 0), stop=(c == nk - 1),
                                skip_group_check=True,
                            )
                            nc.tensor.matmul(
                                den[:, off:QT], ones_sb[:], pt[:, 0:w],
                                start=(c == 0), stop=(c == nk - 1),
                                skip_group_check=True,
                            )
                        rcp = rcp_pool.tile([P, QT], f32, tag="rcp")
                        nc.vector.reciprocal_approx_fast(rcp[:], den[:])
                        nc.vector.tensor_mul(o_sb[:, h, :], po[:], rcp[:])

                        if j == NJ - 1:
                            # last tile: exchange each head as soon as it is
                            # ready, so the final A2A (and its oproj chunk)
                            # is as small and early as possible.
                            nc.sync.dma_start(
                                a2a_in_l[h].rearrange("(d p) t -> p d t", p=P)[
                                    :, bass.ts(b_half, 4), :
                                ],
                                o_sb[:, h, :].rearrange("p (s t) -> p s t", s=4),
                            )
                            nc.gpsimd.collective_compute(
                                "AllToAll",
                                mybir.AluOpType.bypass,
                                replica_groups=GROUPS_A2A,
                                ins=[a2a_in_l[h].opt()],
                                outs=[a2a_out_l[h].opt()],
                            )
                        elif h % 2 == 1:
                            # AllToAll this head-pair's o: shard s carries my
                            # 2 heads x 128 dv for the 128 tokens core s owns.
                            half = h // 2
                            a2a_in_v = a2a_in[j][half].rearrange(
                                "(d hh p) t -> hh p d t", d=8, hh=2, p=P
                            )
                            for hh in range(2):
                                nc.sync.dma_start(
                                    a2a_in_v[hh, :, bass.ts(b_half, 4), :],
                                    o_sb[:, 2 * half + hh, :].rearrange(
                                        "p (s t) -> p s t", s=4
                                    ),
                                )
                            nc.gpsimd.collective_compute(
                                "AllToAll",
                                mybir.AluOpType.bypass,
                                replica_groups=GROUPS_A2A,
                                ins=[a2a_in[j][half].opt()],
                                outs=[a2a_out[j][half].opt()],
                            )

                    # ---- out-projection for tile j-1 (A2A long done) ----
                    if prev_pieces is not None:
                        oproj(j - 1, prev_pieces)
                    if j < NJ - 1:
                        prev_pieces = []
                        for half in range(2):
                            t = orecv_pool.tile([P, 8, P], bf16, tag="orecv")
                            nc.sync.dma_start(
                                t[:],
                                a2a_out[j][half].rearrange(
                                    "(x p) t -> p x t", p=P
                                )[:, bass.ts(b_half, 8), :],
                            )
                            cids = [
                                4 * (i // 2) + 2 * half + (i % 2) for i in range(8)
                            ]
                            prev_pieces.append((t, cids))
                    else:
                        prev_pieces = []
                        for h in range(HG):
                            t = orecv_pool.tile([P, 4, P], bf16, tag="orecvl")
                            nc.sync.dma_start(
                                t[:],
                                a2a_out_l[h].rearrange("(d p) t -> p d t", p=P)[
                                    :, bass.ts(b_half, 4), :
                                ],
                            )
                            prev_pieces.append((t, [4 * s + h for s in range(4)]))

                # last tile: accumulate heads 0-2 while head 3's exchange is
                # still in flight, then only the 4 head-3 chunks trail it
                stg = []
                for n2 in range(N2):
                    ps = ps_mm.tile([P, QT], f32, tag="mm")
                    flat = [
                        (t, i, c)
                        for (t, cids) in prev_pieces[:3]
                        for i, c in enumerate(cids)
                    ]
                    for k, (t, i, c) in enumerate(flat):
                        nc.tensor.matmul(
                            ps[:],
                            t[:, i, :],
                            wot_sb[:, c, bass.ts(n2, QT)],
                            start=(k == 0),
                            stop=(k == len(flat) - 1),
                        )
                    ostg = ostg_pool.tile([P, QT], f32, tag="ostg")
                    nc.vector.tensor_copy(ostg[:], ps[:])
                    stg.append(ostg)
                t3, cids3 = prev_pieces[3]
                for n2 in range(N2):
                    ps = ps_mm.tile([P, QT], f32, tag="mm")
                    for i, c in enumerate(cids3):
                        nc.tensor.matmul(
                            ps[:],
                            t3[:, i, :],
                            wot_sb[:, c, bass.ts(n2, QT)],
                            start=(i == 0),
                            stop=(i == len(cids3) - 1),
                        )
                    nc.vector.tensor_add(stg[n2][:], stg[n2][:], ps[:])
                    nc.sync.dma_start(
                        out_d[bass.ts(NJ - 1, P), bass.ts(n2, QT)], stg[n2][:]
                    )

    nc.compile()
    return nc


def host_prep(x, wq, wk, wv, wo, S):
    """Build the 8 per-core input maps (numpy, bf16)."""
    import ml_dtypes

    bf = ml_dtypes.bfloat16
    perm = np.concatenate(
        [np.arange(0, ROPE, 2), np.arange(1, ROPE, 2), np.arange(ROPE, HEAD_DIM)]
    )
    wq_p = wq.reshape(N_HEADS, HEAD_DIM, D_MODEL)[:, perm, :]
    wk_p = wk.reshape(N_KV, HEAD_DIM, D_MODEL)[:, perm, :]

    inv = THETA ** (-np.arange(0, ROPE, 2, dtype=np.float64) / ROPE)  # [32]
    t = np.arange(S, dtype=np.float64)
    ang = np.outer(inv, t)  # [32, S]
    cosT, sinT = np.cos(ang), np.sin(ang)
    cos = np.ascontiguousarray(np.concatenate([cosT, cosT], 0)).astype(bf)  # [64,S]
    sin = np.ascontiguousarray(np.concatenate([-sinT, sinT], 0)).astype(bf)  # [64,S]

    kk = np.arange(128)[:, None]
    qq = np.arange(128)[None, :]
    mask = (kk <= qq).astype(bf)  # [128,128] fine-diagonal causal mask
    ident = np.eye(128, dtype=bf)

    wot = np.ascontiguousarray(wo.T).astype(bf)  # [D feat, D out]

    in_maps = []
    for c in range(N_CORES):
        b, g = divmod(c, 4)
        xt = np.ascontiguousarray(x[b, :S].T).astype(bf)
        wqt = np.ascontiguousarray(
            wq_p[HG * g : HG * (g + 1)].reshape(GD, D_MODEL).T
        ).astype(bf)
        wkt = np.ascontiguousarray(wk_p[g].T).astype(bf)
        wvt = np.ascontiguousarray(wv[HEAD_DIM * g : HEAD_DIM * (g + 1)].T).astype(bf)
        in_maps.append(
            {
                "xt": xt,
                "wqt": wqt,
                "wkt": wkt,
                "wvt": wvt,
                "wot": wot,
                "cos": cos,
                "sin": sin,
                "mask": mask,
                "ident": ident,
            }
        )
    return in_maps


def run(x, wq, wk, wv, wo, S=None, trace=False):
    from concourse.bass_utils import run_bass_kernel_spmd

    if S is None:
        S = x.shape[1]
    if S not in _BUILD_CACHE:
        _BUILD_CACHE[S] = build_kernel(S)
    nc = _BUILD_CACHE[S]
    in_maps = host_prep(x, wq, wk, wv, wo, S)
    res = run_bass_kernel_spmd(nc, in_maps, core_ids=list(range(N_CORES)), trace=trace)
    out = np.empty((B, S, D_MODEL), np.float32)
    nj = S // 512
    for c in range(N_CORES):
        b, g = divmod(c, 4)
        o = res.results[c]["out"]  # [nj*128, D]: row blk*128+r = global 512*blk+128*g+r
        for blk in range(nj):
            out[b, 512 * blk + 128 * g : 512 * blk + 128 * (g + 1), :] = o[
                128 * blk : 128 * (blk + 1)
            ]
    return out, res


def kernel(x, wq, wk, wv, wo):
    x = np.asarray(x, np.float32)
    wq = np.asarray(wq, np.float32)
    wk = np.asarray(wk, np.float32)
    wv = np.asarray(wv, np.float32)
    wo = np.asarray(wo, np.float32)
    out, _ = run(x, wq, wk, wv, wo)
    return out


# revision 34
# speedup vs baseline: 1.1212x; 1.1212x over previous
"""GroupedQueryAttention Trainium2 kernel (8 NeuronCores, SPMD).

Sharding: core c -> (batch b = c // 4, kv-group g = c % 4).
Each core computes q/k/v projections for its 4 query heads + 1 kv head,
partial-RoPE, causal attention; the per-group attention outputs o are
exchanged with a per-tile AllToAll (each core ends up with the full
2048 o-features for its 128-token slice of the tile), then every core
applies the full out-projection locally -- no post-projection reduction
is needed and only the small pre-projection o travels on the links.

Pipeline (single TileContext, per q-tile j): project(j) -> attend(j)
(AllToAll fires per head-pair as soon as its o rows are normalized) ->
out-project(j-1).  The list scheduler fills PE gaps during the
ACT-bound attention stretches with projection / out-projection matmuls.

All device matmuls run in bf16 (fp32 PSUM accumulation). The host
pre-transposes the operands so the contraction dim lands on SBUF
partitions everywhere with no on-device transposes:
  xt   = x[b].T                  [D, S]
  wqt  = perm(wq)[group].T       [D, 512]   (rows RoPE-deinterleaved)
  wkt  = perm(wk)[group].T       [D, 128]
  wvt  = wv[group].T             [D, 128]
  wot  = wo.T                    [D, D]     (full; o features are global)
The RoPE deinterleave permutation reorders each head's first 64 dims to
[evens, odds]; since q and k use the same permutation, q.k dot products
are unchanged and it never needs undoing.

Causal structure: for the diagonal 128-row k-chunks only the q columns
at-or-right-of the chunk are computed (scores, exp, p@V and the
denominator matmul are all narrowed); the 128x128 block on the fine
diagonal is masked with a single lower-triangular mask.
"""

import math
import sys

sys.path.insert(0, "/opt/trn_rl_repo")

import numpy as np  # noqa: E402

D_MODEL = 2048
N_HEADS = 16
N_KV = 4
HEAD_DIM = 128
ROPE = 64
THETA = 10000.0
B = 2
HG = N_HEADS // N_KV  # 4 query heads per kv group
GD = HG * HEAD_DIM  # 512 o-features per group
N_CORES = 8
GROUPS_A2A = [[0, 1, 2, 3, 4, 5, 6, 7]]

_BUILD_CACHE: dict = {}


def build_kernel(S: int):
    """Build the per-core Bass program for sequence length S (multiple of 512)."""
    import concourse.bass as bass
    import concourse.mybir as mybir
    import concourse.tile as tile
    from concourse import bacc

    assert S % 512 == 0
    P = 128
    QT = 512  # q tile width
    NJ = S // QT  # q tiles
    NO = D_MODEL // P  # contraction chunks for projections (16)
    NS = S // P  # seq chunks of 128
    N2 = D_MODEL // QT  # out-proj column blocks (4)
    bf16 = mybir.dt.bfloat16
    f32 = mybir.dt.float32
    scale = 1.0 / math.sqrt(HEAD_DIM)

    nc = bacc.Bacc(None, target_bir_lowering=False, debug=False, num_devices=N_CORES)

    xt_d = nc.declare_dram_parameter("xt", [D_MODEL, S], bf16, isOutput=False)
    wqt_d = nc.declare_dram_parameter("wqt", [D_MODEL, GD], bf16, isOutput=False)
    wkt_d = nc.declare_dram_parameter("wkt", [D_MODEL, HEAD_DIM], bf16, isOutput=False)
    wvt_d = nc.declare_dram_parameter("wvt", [D_MODEL, HEAD_DIM], bf16, isOutput=False)
    wot_d = nc.declare_dram_parameter("wot", [D_MODEL, D_MODEL], bf16, isOutput=False)
    cos_d = nc.declare_dram_parameter("cos", [ROPE, S], bf16, isOutput=False)
    sin_d = nc.declare_dram_parameter("sin", [ROPE, S], bf16, isOutput=False)
    msk_d = nc.declare_dram_parameter("mask", [P, P], bf16, isOutput=False)
    idn_d = nc.declare_dram_parameter("ident", [P, P], bf16, isOutput=False)
    out_d = nc.declare_dram_parameter("out", [S // 4, D_MODEL], f32, isOutput=True)

    with tile.TileContext(nc) as tc:
        with (
            tc.tile_pool(name="persist", bufs=1) as persist,
            tc.tile_pool(name="dram", bufs=1, space="DRAM") as dram,
        ):
            # ---- persistent SBUF state ----
            k_sb = persist.tile([P, S], bf16)  # kT (rope'd)
            v_sb = persist.tile([P, NS, HEAD_DIM], bf16)  # v natural per chunk
            cos_sb = persist.tile([ROPE, S], bf16)
            sin_sb = persist.tile([ROPE, S], bf16)
            msk_sb = persist.tile([P, P], bf16)
            idn_sb = persist.tile([P, P], bf16)
            ones_sb = persist.tile([P, P], bf16)
            wkt_sb = persist.tile([P, NO, HEAD_DIM], bf16)
            wvt_sb = persist.tile([P, NO, HEAD_DIM], bf16)
            wqt_sb = persist.tile([P, NO, GD], bf16)
            wot_sb = persist.tile([P, NO, D_MODEL], bf16)

            # 8-core AllToAll: 8 shards of [2 heads x 128 dv, 128 tok]; the
            # cross-batch shards carry unread garbage (mesh needs >4 ranks,
            # so the 4-core exchange rides an 8-core op; each core touches
            # only its batch's 1024-row half via a dynamic offset).
            a2a_in = [
                [
                    dram.tile([8 * 2 * P, P], bf16, name=f"a2ai{j}_{h2}")
                    for h2 in range(2)
                ]
                for j in range(NJ)
            ]
            a2a_out = [
                [
                    dram.tile([8 * 2 * P, P], bf16, name=f"a2ao{j}_{h2}")
                    for h2 in range(2)
                ]
                for j in range(NJ)
            ]
            # last tile exchanges per single head (smaller, earlier ops)
            a2a_in_l = [
                dram.tile([8 * P, P], bf16, name=f"a2ail{h}") for h in range(HG)
            ]
            a2a_out_l = [
                dram.tile([8 * P, P], bf16, name=f"a2aol{h}") for h in range(HG)
            ]
            warm_in = dram.tile([8, P], bf16, name="warm_in")
            warm_out = dram.tile([8, P], bf16, name="warm_out")
            b_half = nc.sync.partition_id() // 4  # 0 or 1: my batch group

            # Bulk loads are split between the two HWDGE engines (SP + ACT)
            # and chunked/ordered so the first projection matmuls start as
            # early as possible: the k-projection needs wkt chunk c and xt
            # chunk o in order, everything else can trail.
            nc.vector.memset(ones_sb[:], 1.0)
            wkt_r = wkt_d.rearrange("(o p) m -> p o m", p=P)
            wvt_r = wvt_d.rearrange("(o p) m -> p o m", p=P)
            wqt_r = wqt_d.rearrange("(o p) m -> p o m", p=P)
            wot_r = wot_d.rearrange("(o p) m -> p o m", p=P)
            xt_r = xt_d.rearrange("(o p) s -> p o s", p=P)

            # tiny dummy collective: pays the Mesh first-op setup cost during
            # the weight preload instead of on the first real exchange
            nc.gpsimd.collective_compute(
                "AllToAll",
                mybir.AluOpType.bypass,
                replica_groups=GROUPS_A2A,
                ins=[warm_in.opt()],
                outs=[warm_out.opt()],
            )

            with (
                tc.tile_pool(name="xt_pool", bufs=2) as xt_pool,
                tc.tile_pool(name="q_pool", bufs=2) as q_pool,
                tc.tile_pool(name="o_pool", bufs=2) as o_pool,
                tc.tile_pool(name="vt_pool", bufs=2) as vt_pool,
                tc.tile_pool(name="pt_pool", bufs=4) as pt_pool,
                tc.tile_pool(name="rcp_pool", bufs=2) as rcp_pool,
                tc.tile_pool(name="orecv_pool", bufs=4) as orecv_pool,
                tc.tile_pool(name="ostg_pool", bufs=8) as ostg_pool,
                tc.tile_pool(name="rtmp", bufs=2) as rtmp,
                tc.tile_pool(name="ps_sc", bufs=2, space="PSUM") as ps_sc,
                tc.tile_pool(name="ps_mm", bufs=2, space="PSUM") as ps_mm,
                tc.tile_pool(name="ps_po", bufs=2, space="PSUM") as ps_po,
                tc.tile_pool(name="ps_den", bufs=2, space="PSUM") as ps_den,
            ):

                def load_xt(j):
                    t = xt_pool.tile([P, NO, QT], bf16, tag="xt")
                    for o2 in range(NO // 2):
                        nc.scalar.dma_start(
                            t[:, 2 * o2 : 2 * o2 + 2, :],
                            xt_r[:, 2 * o2 : 2 * o2 + 2, bass.ts(j, QT)],
                        )
                    return t

                def rope(dst, sl, csl):
                    # rotate-half form on deinterleaved rows:
                    #   rows 0:32 = a (even dims), 32:64 = b (odd dims)
                    #   new[0:64] = old[0:64]*cos64 + swap(old[0:64])*sin64
                    # with cos64 = [cosT; cosT], sin64 = [-sinT; sinT].
                    xs = rtmp.tile([64, QT], bf16, tag="xs")
                    nc.vector.tensor_copy(xs[0:32, :], dst[32:64, sl])
                    nc.vector.tensor_copy(xs[32:64, :], dst[0:32, sl])
                    t = rtmp.tile([64, QT], bf16, tag="t")
                    u = rtmp.tile([64, QT], bf16, tag="u")
                    nc.vector.tensor_mul(t[:], xs[:], sin_sb[:, csl])
                    nc.vector.tensor_mul(u[:], dst[0:64, sl], cos_sb[:, csl])
                    nc.vector.tensor_add(dst[0:64, sl], u[:], t[:])

                def oproj(j, pieces):
                    # pieces: [(recv_tile, [global chunk ids])]
                    # out rows [128j, 128j+128) = tokens 512j+128g..  (host maps)
                    flat = [
                        (t, i, c)
                        for (t, cids) in pieces
                        for i, c in enumerate(cids)
                    ]
                    for n2 in range(N2):
                        ps = ps_mm.tile([P, QT], f32, tag="mm")
                        for k, (t, i, c) in enumerate(flat):
                            nc.tensor.matmul(
                                ps[:],
                                t[:, i, :],
                                wot_sb[:, c, bass.ts(n2, QT)],
                                start=(k == 0),
                                stop=(k == len(flat) - 1),
                            )
                        ostg = ostg_pool.tile([P, QT], f32, tag="ostg")
                        nc.vector.tensor_copy(ostg[:], ps[:])
                        nc.sync.dma_start(
                            out_d[bass.ts(j, P), bass.ts(n2, QT)], ostg[:]
                        )

                # ---- startup loads, interleaved across both HWDGE engines so
                # the k-projection's chunk-o inputs land in consumption order
                xts = [None] * NJ
                xts[0] = xt_pool.tile([P, NO, QT], bf16, name="xt0", tag="xt")
                for o in range(NO):
                    if o % 4 == 0:
                        nc.scalar.dma_start(
                            wkt_sb[:, o : o + 4, :], wkt_r[:, o : o + 4, :]
                        )
                    eng = nc.scalar if o % 2 == 0 else nc.sync
                    eng.dma_start(xts[0][:, o, :], xt_r[:, o, bass.ts(0, QT)])
                for c in range(4):
                    nc.scalar.dma_start(
                        wvt_sb[:, 4 * c : 4 * c + 4, :],
                        wvt_r[:, 4 * c : 4 * c + 4, :],
                    )
                for o2 in range(NO // 2):
                    nc.sync.dma_start(
                        wqt_sb[:, 2 * o2 : 2 * o2 + 2, :],
                        wqt_r[:, 2 * o2 : 2 * o2 + 2, :],
                    )
                nc.scalar.dma_start(cos_sb[:], cos_d[:])
                nc.scalar.dma_start(sin_sb[:], sin_d[:])
                nc.scalar.dma_start(idn_sb[:], idn_d[:])
                nc.scalar.dma_start(msk_sb[:], msk_d[:])
                for o in range(NO):
                    nc.sync.dma_start(wot_sb[:, o, :], wot_r[:, o, :])
                all_pieces = [None] * NJ

                for j in range(NJ):
                    jsl = bass.ts(j, QT)
                    if j + 1 < NJ:
                        xts[j + 1] = load_xt(j + 1)
                    xt_sb = xts[j]

                    # ---- projections (+RoPE) for tile j ----
                    ps = ps_mm.tile([P, QT], f32, tag="mm")
                    for o in range(NO):
                        nc.tensor.matmul(
                            ps[:], wkt_sb[:, o, :], xt_sb[:, o, :],
                            start=(o == 0), stop=(o == NO - 1),
                        )
                    nc.vector.tensor_copy(k_sb[:, jsl], ps[:])
                    rope(k_sb, jsl, jsl)

                    ps = ps_mm.tile([P, QT], f32, tag="mm")
                    for o in range(NO):
                        nc.tensor.matmul(
                            ps[:], wvt_sb[:, o, :], xt_sb[:, o, :],
                            start=(o == 0), stop=(o == NO - 1),
                        )
                    vt_sb = vt_pool.tile([P, QT], bf16, tag="vt")
                    nc.vector.tensor_copy(vt_sb[:], ps[:])
                    for i in range(QT // P):
                        # PE transpose-mode: [dv, tok] -> [tok, dv], no DMA
                        tps = ps_mm.tile([P, P], bf16, tag="mm")
                        nc.tensor.transpose(
                            tps[:], vt_sb[:, bass.ts(i, P)], idn_sb[:]
                        )
                        nc.vector.tensor_copy(v_sb[:, 4 * j + i, :], tps[:])

                    q_sb = q_pool.tile([P, HG, QT], bf16, tag="q")
                    for h in range(HG):
                        ps = ps_mm.tile([P, QT], f32, tag="mm")
                        for o in range(NO):
                            nc.tensor.matmul(
                                ps[:], wqt_sb[:, o, bass.ts(h, P)], xt_sb[:, o, :],
                                start=(o == 0), stop=(o == NO - 1),
                            )
                        nc.vector.tensor_copy(q_sb[:, h, :], ps[:])
                        rope(q_sb[:, h, :], slice(0, QT), jsl)

                    # out-projection lags 2 tiles: the CC-init barrier makes
                    # early exchanges slow and very variable; committing the
                    # in-order PE stream to oproj only 2 tiles later keeps it
                    # from head-of-line blocking on a late AllToAll.
                    if j >= 2:
                        oproj(j - 2, all_pieces[j - 2])

                    # ---- attention for tile j ----
                    o_sb = o_pool.tile([P, HG, QT], bf16, tag="o")
                    for h in range(HG):
                        po = ps_po.tile([P, QT], f32, tag="po")
                        den = ps_den.tile([P, QT], f32, tag="den")
                        nk = 4 * (j + 1)
                        for c in range(nk):
                            r = c - 4 * j  # >=0 on the diagonal block
                            off = P * r if r >= 0 else 0
                            w = QT - off
                            sc = ps_sc.tile([P, QT], f32, tag="sc")
                            nc.tensor.matmul(
                                sc[:, 0:w],
                                k_sb[:, bass.ts(c, P)],
                                q_sb[:, h, off:QT],
                                start=True, stop=True,
                            )
                            pt = pt_pool.tile([P, QT], bf16, tag="pt")
                            nc.scalar.activation(
                                pt[:, 0:w], sc[:, 0:w],
                                mybir.ActivationFunctionType.Exp,
                                scale=scale,
                            )
                            if r >= 0:
                                nc.vector.tensor_mul(
                                    pt[:, 0:P], pt[:, 0:P], msk_sb[:]
                                )
                            nc.tensor.matmul(
                                po[:, off:QT], v_sb[:, c, :], pt[:, 0:w],
                                start=(c == 0), stop=(c == nk - 1),
                                skip_group_check=True,
                            )
                            nc.tensor.matmul(
                                den[:, off:QT], ones_sb[:], pt[:, 0:w],
                                start=(c == 0), stop=(c == nk - 1),
                                skip_group_check=True,
                            )
                        rcp = rcp_pool.tile([P, QT], f32, tag="rcp")
                        nc.vector.reciprocal_approx_fast(rcp[:], den[:])
                        nc.vector.tensor_mul(o_sb[:, h, :], po[:], rcp[:])

                        if j == NJ - 1:
                            # last tile: exchange each head as soon as it is
                            # ready, so the final A2A (and its oproj chunk)
                            # is as small and early as possible.
                            nc.sync.dma_start(
                                a2a_in_l[h].rearrange("(d p) t -> p d t", p=P)[
                                    :, bass.ts(b_half, 4), :
                                ],
                                o_sb[:, h, :].rearrange("p (s t) -> p s t", s=4),
                            )
                            nc.gpsimd.collective_compute(
                                "AllToAll",
                                mybir.AluOpType.bypass,
                                replica_groups=GROUPS_A2A,
                                ins=[a2a_in_l[h].opt()],
                                outs=[a2a_out_l[h].opt()],
                            )
                        elif h % 2 == 1:
                            # AllToAll this head-pair's o: shard s carries my
                            # 2 heads x 128 dv for the 128 tokens core s owns.
                            half = h // 2
                            a2a_in_v = a2a_in[j][half].rearrange(
                                "(d hh p) t -> hh p d t", d=8, hh=2, p=P
                            )
                            for hh in range(2):
                                nc.sync.dma_start(
                                    a2a_in_v[hh, :, bass.ts(b_half, 4), :],
                                    o_sb[:, 2 * half + hh, :].rearrange(
                                        "p (s t) -> p s t", s=4
                                    ),
                                )
                            nc.gpsimd.collective_compute(
                                "AllToAll",
                                mybir.AluOpType.bypass,
                                replica_groups=GROUPS_A2A,
                                ins=[a2a_in[j][half].opt()],
                                outs=[a2a_out[j][half].opt()],
                            )

                    if j < NJ - 1:
                        prev_pieces = []
                        for half in range(2):
                            t = orecv_pool.tile([P, 8, P], bf16, tag="orecv")
                            nc.sync.dma_start(
                                t[:],
                                a2a_out[j][half].rearrange(
                                    "(x p) t -> p x t", p=P
                                )[:, bass.ts(b_half, 8), :],
                            )
                            cids = [
                                4 * (i // 2) + 2 * half + (i % 2) for i in range(8)
                            ]
                            prev_pieces.append((t, cids))
                    else:
                        prev_pieces = []
                        for h in range(HG):
                            t = orecv_pool.tile([P, 4, P], bf16, tag="orecvl")
                            nc.sync.dma_start(
                                t[:],
                                a2a_out_l[h].rearrange("(d p) t -> p d t", p=P)[
                                    :, bass.ts(b_half, 4), :
                                ],
                            )
                            prev_pieces.append((t, [4 * s + h for s in range(4)]))
                    all_pieces[j] = prev_pieces

                oproj(NJ - 2, all_pieces[NJ - 2])
                prev_pieces = all_pieces[NJ - 1]
                # last tile: accumulate heads 0-2 while head 3's exchange is
                # still in flight, then only the 4 head-3 chunks trail it
                stg = []
                for n2 in range(N2):
                    ps = ps_mm.tile([P, QT], f32, tag="mm")
                    flat = [
                        (t, i, c)
                        for (t, cids) in prev_pieces[:3]
                        for i, c in enumerate(cids)
                    ]
                    for k, (t, i, c) in enumerate(flat):
                        nc.tensor.matmul(
                            ps[:],
                            t[:, i, :],
                            wot_sb[:, c, bass.ts(n2, QT)],
                            start=(k == 0),
                            stop=(k == len(flat) - 1),
                        )
                    ostg = ostg_pool.tile([P, QT], f32, tag="ostg")
                    nc.vector.tensor_copy(ostg[:], ps[:])
                    stg.append(ostg)
                t3, cids3 = prev_pieces[3]
                for n2 in range(N2):
                    ps = ps_mm.tile([P, QT], f32, tag="mm")
                    for i, c in enumerate(cids3):
                        nc.tensor.matmul(
                            ps[:],
                            t3[:, i, :],
                            wot_sb[:, c, bass.ts(n2, QT)],
                            start=(i == 0),
                            stop=(i == len(cids3) - 1),
                        )
                    nc.vector.tensor_add(stg[n2][:], stg[n2][:], ps[:])
                    nc.sync.dma_start(
                        out_d[bass.ts(NJ - 1, P), bass.ts(n2, QT)], stg[n2][:]
                    )

    nc.compile()
    return nc


def host_prep(x, wq, wk, wv, wo, S):
    """Build the 8 per-core input maps (numpy, bf16)."""
    import ml_dtypes

    bf = ml_dtypes.bfloat16
    perm = np.concatenate(
        [np.arange(0, ROPE, 2), np.arange(1, ROPE, 2), np.arange(ROPE, HEAD_DIM)]
    )
    wq_p = wq.reshape(N_HEADS, HEAD_DIM, D_MODEL)[:, perm, :]
    wk_p = wk.reshape(N_KV, HEAD_DIM, D_MODEL)[:, perm, :]

    inv = THETA ** (-np.arange(0, ROPE, 2, dtype=np.float64) / ROPE)  # [32]
    t = np.arange(S, dtype=np.float64)
    ang = np.outer(inv, t)  # [32, S]
    cosT, sinT = np.cos(ang), np.sin(ang)
    cos = np.ascontiguousarray(np.concatenate([cosT, cosT], 0)).astype(bf)  # [64,S]
    sin = np.ascontiguousarray(np.concatenate([-sinT, sinT], 0)).astype(bf)  # [64,S]

    kk = np.arange(128)[:, None]
    qq = np.arange(128)[None, :]
    mask = (kk <= qq).astype(bf)  # [128,128] fine-diagonal causal mask
    ident = np.eye(128, dtype=bf)

    wot = np.ascontiguousarray(wo.T).astype(bf)  # [D feat, D out]

    in_maps = []
    for c in range(N_CORES):
        b, g = divmod(c, 4)
        xt = np.ascontiguousarray(x[b, :S].T).astype(bf)
        wqt = np.ascontiguousarray(
            wq_p[HG * g : HG * (g + 1)].reshape(GD, D_MODEL).T
        ).astype(bf)
        wkt = np.ascontiguousarray(wk_p[g].T).astype(bf)
        wvt = np.ascontiguousarray(wv[HEAD_DIM * g : HEAD_DIM * (g + 1)].T).astype(bf)
        in_maps.append(
            {
                "xt": xt,
                "wqt": wqt,
                "wkt": wkt,
                "wvt": wvt,
                "wot": wot,
                "cos": cos,
                "sin": sin,
                "mask": mask,
                "ident": ident,
            }
        )
    return in_maps


def run(x, wq, wk, wv, wo, S=None, trace=False):
    from concourse.bass_utils import run_bass_kernel_spmd

    if S is None:
        S = x.shape[1]
    if S not in _BUILD_CACHE:
        _BUILD_CACHE[S] = build_kernel(S)
    nc = _BUILD_CACHE[S]
    in_maps = host_prep(x, wq, wk, wv, wo, S)
    res = run_bass_kernel_spmd(nc, in_maps, core_ids=list(range(N_CORES)), trace=trace)
    out = np.empty((B, S, D_MODEL), np.float32)
    nj = S // 512
    for c in range(N_CORES):
        b, g = divmod(c, 4)
        o = res.results[c]["out"]  # [nj*128, D]: row blk*128+r = global 512*blk+128*g+r
        for blk in range(nj):
            out[b, 512 * blk + 128 * g : 512 * blk + 128 * (g + 1), :] = o[
                128 * blk : 128 * (blk + 1)
            ]
    return out, res


def kernel(x, wq, wk, wv, wo):
    x = np.asarray(x, np.float32)
    wq = np.asarray(wq, np.float32)
    wk = np.asarray(wk, np.float32)
    wv = np.asarray(wv, np.float32)
    wo = np.asarray(wo, np.float32)
    out, _ = run(x, wq, wk, wv, wo)
    return out


# revision 41
# speedup vs baseline: 1.1407x; 1.0174x over previous
"""GroupedQueryAttention Trainium2 kernel (8 NeuronCores, SPMD).

Sharding: core c -> (batch b = c // 4, kv-group g = c % 4).
Each core computes q/k/v projections for its 4 query heads + 1 kv head,
partial-RoPE, causal attention; the per-group attention outputs o are
exchanged with a per-tile AllToAll (each core ends up with the full
2048 o-features for its 128-token slice of the tile), then every core
applies the full out-projection locally -- no post-projection reduction
is needed and only the small pre-projection o travels on the links.

Pipeline (single TileContext, per q-tile j): project(j) -> attend(j)
(AllToAll fires per head-pair as soon as its o rows are normalized) ->
out-project(j-1).  The list scheduler fills PE gaps during the
ACT-bound attention stretches with projection / out-projection matmuls.

All device matmuls run in bf16 (fp32 PSUM accumulation). The host
pre-transposes the operands so the contraction dim lands on SBUF
partitions everywhere with no on-device transposes:
  xt   = x[b].T                  [D, S]
  wqt  = perm(wq)[group].T       [D, 512]   (rows RoPE-deinterleaved)
  wkt  = perm(wk)[group].T       [D, 128]
  wvt  = wv[group].T             [D, 128]
  wot  = wo.T                    [D, D]     (full; o features are global)
The RoPE deinterleave permutation reorders each head's first 64 dims to
[evens, odds]; since q and k use the same permutation, q.k dot products
are unchanged and it never needs undoing.

Causal structure: for the diagonal 128-row k-chunks only the q columns
at-or-right-of the chunk are computed (scores, exp, p@V and the
denominator matmul are all narrowed); the 128x128 block on the fine
diagonal is masked with a single lower-triangular mask.
"""

import math
import sys

sys.path.insert(0, "/opt/trn_rl_repo")

import numpy as np  # noqa: E402

D_MODEL = 2048
N_HEADS = 16
N_KV = 4
HEAD_DIM = 128
ROPE = 64
THETA = 10000.0
B = 2
HG = N_HEADS // N_KV  # 4 query heads per kv group
GD = HG * HEAD_DIM  # 512 o-features per group
N_CORES = 8
GROUPS_A2A = [[0, 1, 2, 3, 4, 5, 6, 7]]
GROUPS_AG = [[0, 1, 2, 3], [4, 5, 6, 7]]

_BUILD_CACHE: dict = {}


def build_kernel(S: int):
    """Build the per-core Bass program for sequence length S (multiple of 512)."""
    import concourse.bass as bass
    import concourse.mybir as mybir
    import concourse.tile as tile
    from concourse import bacc

    assert S % 512 == 0
    P = 128
    QT = 512  # q tile width
    NJ = S // QT  # q tiles
    NO = D_MODEL // P  # contraction chunks for projections (16)
    NS = S // P  # seq chunks of 128
    N2 = D_MODEL // QT  # out-proj column blocks (4)
    bf16 = mybir.dt.bfloat16
    f32 = mybir.dt.float32
    scale = 1.0 / math.sqrt(HEAD_DIM)

    nc = bacc.Bacc(None, target_bir_lowering=False, debug=False, num_devices=N_CORES)

    xt_d = nc.declare_dram_parameter("xt", [D_MODEL, S], bf16, isOutput=False)
    wqt_d = nc.declare_dram_parameter("wqt", [D_MODEL, GD], bf16, isOutput=False)
    wkt_d = nc.declare_dram_parameter("wkt", [D_MODEL, HEAD_DIM], bf16, isOutput=False)
    wvt_d = nc.declare_dram_parameter("wvt", [D_MODEL, HEAD_DIM], bf16, isOutput=False)
    wot_d = nc.declare_dram_parameter("wot", [D_MODEL, D_MODEL], bf16, isOutput=False)
    cos_d = nc.declare_dram_parameter("cos", [ROPE, S], bf16, isOutput=False)
    sin_d = nc.declare_dram_parameter("sin", [ROPE, S], bf16, isOutput=False)
    msk_d = nc.declare_dram_parameter("mask", [P, P], bf16, isOutput=False)
    idn_d = nc.declare_dram_parameter("ident", [P, P], bf16, isOutput=False)
    out_d = nc.declare_dram_parameter("out", [S // 4, D_MODEL], f32, isOutput=True)

    with tile.TileContext(nc) as tc:
        with (
            tc.tile_pool(name="persist", bufs=1) as persist,
            tc.tile_pool(name="dram", bufs=1, space="DRAM") as dram,
        ):
            # ---- persistent SBUF state ----
            k_sb = persist.tile([P, S], bf16)  # kT (rope'd)
            v_sb = persist.tile([P, NS, HEAD_DIM], bf16)  # v natural per chunk
            cos_sb = persist.tile([ROPE, S], bf16)
            sin_sb = persist.tile([ROPE, S], bf16)
            msk_sb = persist.tile([P, P], bf16)
            idn_sb = persist.tile([P, P], bf16)
            ones_sb = persist.tile([P, P], bf16)
            wkt_sb = persist.tile([P, NO, HEAD_DIM], bf16)
            wvt_sb = persist.tile([P, NO, HEAD_DIM], bf16)
            wqt_sb = persist.tile([P, NO, GD], bf16)
            wot_sb = persist.tile([P, NO, D_MODEL], bf16)

            # 8-core AllToAll: 8 shards of [2 heads x 128 dv, 128 tok]; the
            # cross-batch shards carry unread garbage (mesh needs >4 ranks,
            # so the 4-core exchange rides an 8-core op; each core touches
            # only its batch's 1024-row half via a dynamic offset).
            a2a_in = [
                [
                    dram.tile([8 * 2 * P, P], bf16, name=f"a2ai{j}_{h2}")
                    for h2 in range(2)
                ]
                for j in range(NJ)
            ]
            a2a_out = [
                [
                    dram.tile([8 * 2 * P, P], bf16, name=f"a2ao{j}_{h2}")
                    for h2 in range(2)
                ]
                for j in range(NJ)
            ]
            # last tile: per-head 4-core AllGather (lower floor than the
            # 8-core mesh A2A; latency is all that matters at the tail)
            ag_in_l = [
                dram.tile([P, QT], bf16, name=f"agil{h}") for h in range(HG)
            ]
            ag_out_l = [
                dram.tile([4 * P, QT], bf16, name=f"agol{h}") for h in range(HG)
            ]
            warm_in = dram.tile([8, P], bf16, name="warm_in")
            warm_out = dram.tile([8, P], bf16, name="warm_out")
            b_half = nc.sync.partition_id() // 4  # 0 or 1: my batch group
            g_idx = nc.sync.partition_id() % 4  # my kv-group within the batch

            # Bulk loads are split between the two HWDGE engines (SP + ACT)
            # and chunked/ordered so the first projection matmuls start as
            # early as possible: the k-projection needs wkt chunk c and xt
            # chunk o in order, everything else can trail.
            nc.vector.memset(ones_sb[:], 1.0)
            wkt_r = wkt_d.rearrange("(o p) m -> p o m", p=P)
            wvt_r = wvt_d.rearrange("(o p) m -> p o m", p=P)
            wqt_r = wqt_d.rearrange("(o p) m -> p o m", p=P)
            wot_r = wot_d.rearrange("(o p) m -> p o m", p=P)
            xt_r = xt_d.rearrange("(o p) s -> p o s", p=P)

            # tiny dummy collective: pays the Mesh first-op setup cost during
            # the weight preload instead of on the first real exchange
            nc.gpsimd.collective_compute(
                "AllToAll",
                mybir.AluOpType.bypass,
                replica_groups=GROUPS_A2A,
                ins=[warm_in.opt()],
                outs=[warm_out.opt()],
            )

            with (
                tc.tile_pool(name="xt_pool", bufs=2) as xt_pool,
                tc.tile_pool(name="q_pool", bufs=2) as q_pool,
                tc.tile_pool(name="o_pool", bufs=2) as o_pool,
                tc.tile_pool(name="vt_pool", bufs=2) as vt_pool,
                tc.tile_pool(name="pt_pool", bufs=4) as pt_pool,
                tc.tile_pool(name="rcp_pool", bufs=2) as rcp_pool,
                tc.tile_pool(name="orecv_pool", bufs=4) as orecv_pool,
                tc.tile_pool(name="ostg_pool", bufs=8) as ostg_pool,
                tc.tile_pool(name="rtmp", bufs=2) as rtmp,
                tc.tile_pool(name="ps_sc", bufs=2, space="PSUM") as ps_sc,
                tc.tile_pool(name="ps_mm", bufs=2, space="PSUM") as ps_mm,
                tc.tile_pool(name="ps_po", bufs=2, space="PSUM") as ps_po,
                tc.tile_pool(name="ps_den", bufs=2, space="PSUM") as ps_den,
            ):

                def load_xt(j):
                    t = xt_pool.tile([P, NO, QT], bf16, tag="xt")
                    for o2 in range(NO // 2):
                        nc.scalar.dma_start(
                            t[:, 2 * o2 : 2 * o2 + 2, :],
                            xt_r[:, 2 * o2 : 2 * o2 + 2, bass.ts(j, QT)],
                        )
                    return t

                def rope(dst, sl, csl):
                    # rotate-half form on deinterleaved rows:
                    #   rows 0:32 = a (even dims), 32:64 = b (odd dims)
                    #   new[0:64] = old[0:64]*cos64 + swap(old[0:64])*sin64
                    # with cos64 = [cosT; cosT], sin64 = [-sinT; sinT].
                    xs = rtmp.tile([64, QT], bf16, tag="xs")
                    nc.vector.tensor_copy(xs[0:32, :], dst[32:64, sl])
                    nc.vector.tensor_copy(xs[32:64, :], dst[0:32, sl])
                    t = rtmp.tile([64, QT], bf16, tag="t")
                    u = rtmp.tile([64, QT], bf16, tag="u")
                    nc.vector.tensor_mul(t[:], xs[:], sin_sb[:, csl])
                    nc.vector.tensor_mul(u[:], dst[0:64, sl], cos_sb[:, csl])
                    nc.vector.tensor_add(dst[0:64, sl], u[:], t[:])

                def oproj(j, pieces):
                    # pieces: [(recv_tile, [global chunk ids])]
                    # out rows [128j, 128j+128) = tokens 512j+128g..  (host maps)
                    flat = [
                        (t, i, c)
                        for (t, cids) in pieces
                        for i, c in enumerate(cids)
                    ]
                    for n2 in range(N2):
                        ps = ps_mm.tile([P, QT], f32, tag="mm")
                        for k, (t, i, c) in enumerate(flat):
                            nc.tensor.matmul(
                                ps[:],
                                t[:, i, :],
                                wot_sb[:, c, bass.ts(n2, QT)],
                                start=(k == 0),
                                stop=(k == len(flat) - 1),
                            )
                        ostg = ostg_pool.tile([P, QT], f32, tag="ostg")
                        nc.vector.tensor_copy(ostg[:], ps[:])
                        nc.sync.dma_start(
                            out_d[bass.ts(j, P), bass.ts(n2, QT)], ostg[:]
                        )

                # ---- startup loads, interleaved across both HWDGE engines so
                # the k-projection's chunk-o inputs land in consumption order
                xts = [None] * NJ
                xts[0] = xt_pool.tile([P, NO, QT], bf16, name="xt0", tag="xt")
                for o in range(NO):
                    if o % 4 == 0:
                        nc.scalar.dma_start(
                            wkt_sb[:, o : o + 4, :], wkt_r[:, o : o + 4, :]
                        )
                    eng = nc.scalar if o % 2 == 0 else nc.sync
                    eng.dma_start(xts[0][:, o, :], xt_r[:, o, bass.ts(0, QT)])
                for c in range(4):
                    nc.scalar.dma_start(
                        wvt_sb[:, 4 * c : 4 * c + 4, :],
                        wvt_r[:, 4 * c : 4 * c + 4, :],
                    )
                # wqt by column halves (heads 0-1 then 2-3) so the first
                # q-head projections can start before the whole load lands
                for c in range(4):
                    nc.scalar.dma_start(
                        wqt_sb[:, 4 * c : 4 * c + 4, 0 : 2 * P],
                        wqt_r[:, 4 * c : 4 * c + 4, 0 : 2 * P],
                    )
                    nc.sync.dma_start(
                        wqt_sb[:, 4 * c : 4 * c + 4, 2 * P : 4 * P],
                        wqt_r[:, 4 * c : 4 * c + 4, 2 * P : 4 * P],
                    )
                nc.scalar.dma_start(cos_sb[:], cos_d[:])
                nc.scalar.dma_start(sin_sb[:], sin_d[:])
                nc.scalar.dma_start(idn_sb[:], idn_d[:])
                nc.scalar.dma_start(msk_sb[:], msk_d[:])
                for o in range(NO):
                    nc.sync.dma_start(wot_sb[:, o, :], wot_r[:, o, :])
                all_pieces = [None] * NJ

                for j in range(NJ):
                    jsl = bass.ts(j, QT)
                    if j + 1 < NJ:
                        xts[j + 1] = load_xt(j + 1)
                    xt_sb = xts[j]

                    # ---- projections (+RoPE) for tile j ----
                    ps = ps_mm.tile([P, QT], f32, tag="mm")
                    for o in range(NO):
                        nc.tensor.matmul(
                            ps[:], wkt_sb[:, o, :], xt_sb[:, o, :],
                            start=(o == 0), stop=(o == NO - 1),
                        )
                    nc.vector.tensor_copy(k_sb[:, jsl], ps[:])
                    rope(k_sb, jsl, jsl)

                    ps = ps_mm.tile([P, QT], f32, tag="mm")
                    for o in range(NO):
                        nc.tensor.matmul(
                            ps[:], wvt_sb[:, o, :], xt_sb[:, o, :],
                            start=(o == 0), stop=(o == NO - 1),
                        )
                    vt_sb = vt_pool.tile([P, QT], bf16, tag="vt")
                    nc.vector.tensor_copy(vt_sb[:], ps[:])
                    for i in range(QT // P):
                        # PE transpose-mode: [dv, tok] -> [tok, dv], no DMA
                        tps = ps_mm.tile([P, P], bf16, tag="mm")
                        nc.tensor.transpose(
                            tps[:], vt_sb[:, bass.ts(i, P)], idn_sb[:]
                        )
                        nc.vector.tensor_copy(v_sb[:, 4 * j + i, :], tps[:])

                    q_sb = q_pool.tile([P, HG, QT], bf16, tag="q")
                    for h in range(HG):
                        ps = ps_mm.tile([P, QT], f32, tag="mm")
                        for o in range(NO):
                            nc.tensor.matmul(
                                ps[:], wqt_sb[:, o, bass.ts(h, P)], xt_sb[:, o, :],
                                start=(o == 0), stop=(o == NO - 1),
                            )
                        nc.vector.tensor_copy(q_sb[:, h, :], ps[:])
                        rope(q_sb[:, h, :], slice(0, QT), jsl)

                    # out-projection lags 2 tiles: the CC-init barrier makes
                    # early exchanges slow and very variable; committing the
                    # in-order PE stream to oproj only 2 tiles later keeps it
                    # from head-of-line blocking on a late AllToAll.
                    if j >= 2:
                        oproj(j - 2, all_pieces[j - 2])

                    # ---- attention for tile j ----
                    o_sb = o_pool.tile([P, HG, QT], bf16, tag="o")
                    for h in range(HG):
                        po = ps_po.tile([P, QT], f32, tag="po")
                        den = ps_den.tile([P, QT], f32, tag="den")
                        nk = 4 * (j + 1)
                        for c in range(nk):
                            r = c - 4 * j  # >=0 on the diagonal block
                            off = P * r if r >= 0 else 0
                            w = QT - off
                            sc = ps_sc.tile([P, QT], f32, tag="sc")
                            nc.tensor.matmul(
                                sc[:, 0:w],
                                k_sb[:, bass.ts(c, P)],
                                q_sb[:, h, off:QT],
                                start=True, stop=True,
                            )
                            pt = pt_pool.tile([P, QT], bf16, tag="pt")
                            nc.scalar.activation(
                                pt[:, 0:w], sc[:, 0:w],
                                mybir.ActivationFunctionType.Exp,
                                scale=scale,
                            )
                            if r >= 0:
                                nc.vector.tensor_mul(
                                    pt[:, 0:P], pt[:, 0:P], msk_sb[:]
                                )
                            nc.tensor.matmul(
                                po[:, off:QT], v_sb[:, c, :], pt[:, 0:w],
                                start=(c == 0), stop=(c == nk - 1),
                                skip_group_check=True,
                            )
                            nc.tensor.matmul(
                                den[:, off:QT], ones_sb[:], pt[:, 0:w],
                                start=(c == 0), stop=(c == nk - 1),
                                skip_group_check=True,
                            )
                        rcp = rcp_pool.tile([P, QT], f32, tag="rcp")
                        nc.vector.reciprocal_approx_fast(rcp[:], den[:])
                        nc.vector.tensor_mul(o_sb[:, h, :], po[:], rcp[:])

                        if j == NJ - 1:
                            # last tile: exchange each head as soon as it is
                            # ready, so the final op (and its oproj chunk)
                            # is as small and early as possible.
                            nc.sync.dma_start(ag_in_l[h][:], o_sb[:, h, :])
                            nc.gpsimd.collective_compute(
                                "AllGather",
                                mybir.AluOpType.bypass,
                                replica_groups=GROUPS_AG,
                                ins=[ag_in_l[h].opt()],
                                outs=[ag_out_l[h].opt()],
                            )
                        elif h % 2 == 1:
                            # AllToAll this head-pair's o: shard s carries my
                            # 2 heads x 128 dv for the 128 tokens core s owns.
                            half = h // 2
                            a2a_in_v = a2a_in[j][half].rearrange(
                                "(d hh p) t -> hh p d t", d=8, hh=2, p=P
                            )
                            for hh in range(2):
                                nc.sync.dma_start(
                                    a2a_in_v[hh, :, bass.ts(b_half, 4), :],
                                    o_sb[:, 2 * half + hh, :].rearrange(
                                        "p (s t) -> p s t", s=4
                                    ),
                                )
                            nc.gpsimd.collective_compute(
                                "AllToAll",
                                mybir.AluOpType.bypass,
                                replica_groups=GROUPS_A2A,
                                ins=[a2a_in[j][half].opt()],
                                outs=[a2a_out[j][half].opt()],
                            )

                    if j < NJ - 1:
                        prev_pieces = []
                        for half in range(2):
                            t = orecv_pool.tile([P, 8, P], bf16, tag="orecv")
                            nc.sync.dma_start(
                                t[:],
                                a2a_out[j][half].rearrange(
                                    "(x p) t -> p x t", p=P
                                )[:, bass.ts(b_half, 8), :],
                            )
                            cids = [
                                4 * (i // 2) + 2 * half + (i % 2) for i in range(8)
                            ]
                            prev_pieces.append((t, cids))
                    else:
                        prev_pieces = []
                        for h in range(HG):
                            t = orecv_pool.tile([P, 4, P], bf16, tag="orecvl")
                            nc.sync.dma_start(
                                t[:],
                                ag_out_l[h].rearrange("(s p) t -> p s t", p=P)[
                                    :, :, bass.ts(g_idx, P)
                                ],
                            )
                            prev_pieces.append((t, [4 * s + h for s in range(4)]))
                    all_pieces[j] = prev_pieces

                oproj(NJ - 2, all_pieces[NJ - 2])
                prev_pieces = all_pieces[NJ - 1]
                # last tile: accumulate heads 0-2 while head 3's exchange is
                # still in flight, then only the 4 head-3 chunks trail it
                stg = []
                for n2 in range(N2):
                    ps = ps_mm.tile([P, QT], f32, tag="mm")
                    flat = [
                        (t, i, c)
                        for (t, cids) in prev_pieces[:3]
                        for i, c in enumerate(cids)
                    ]
                    for k, (t, i, c) in enumerate(flat):
                        nc.tensor.matmul(
                            ps[:],
                            t[:, i, :],
                            wot_sb[:, c, bass.ts(n2, QT)],
                            start=(k == 0),
                            stop=(k == len(flat) - 1),
                        )
                    ostg = ostg_pool.tile([P, QT], f32, tag="ostg")
                    nc.vector.tensor_copy(ostg[:], ps[:])
                    stg.append(ostg)
                t3, cids3 = prev_pieces[3]
                for n2 in range(N2):
                    ps = ps_mm.tile([P, QT], f32, tag="mm")
                    for i, c in enumerate(cids3):
                        nc.tensor.matmul(
                            ps[:],
                            t3[:, i, :],
                            wot_sb[:, c, bass.ts(n2, QT)],
                            start=(i == 0),
                            stop=(i == len(cids3) - 1),
                        )
                    nc.vector.tensor_add(stg[n2][:], stg[n2][:], ps[:])
                    nc.sync.dma_start(
                        out_d[bass.ts(NJ - 1, P), bass.ts(n2, QT)], stg[n2][:]
                    )

    nc.compile()
    return nc


def host_prep(x, wq, wk, wv, wo, S):
    """Build the 8 per-core input maps (numpy, bf16)."""
    import ml_dtypes

    bf = ml_dtypes.bfloat16
    perm = np.concatenate(
        [np.arange(0, ROPE, 2), np.arange(1, ROPE, 2), np.arange(ROPE, HEAD_DIM)]
    )
    wq_p = wq.reshape(N_HEADS, HEAD_DIM, D_MODEL)[:, perm, :]
    wk_p = wk.reshape(N_KV, HEAD_DIM, D_MODEL)[:, perm, :]

    inv = THETA ** (-np.arange(0, ROPE, 2, dtype=np.float64) / ROPE)  # [32]
    t = np.arange(S, dtype=np.float64)
    ang = np.outer(inv, t)  # [32, S]
    cosT, sinT = np.cos(ang), np.sin(ang)
    cos = np.ascontiguousarray(np.concatenate([cosT, cosT], 0)).astype(bf)  # [64,S]
    sin = np.ascontiguousarray(np.concatenate([-sinT, sinT], 0)).astype(bf)  # [64,S]

    kk = np.arange(128)[:, None]
    qq = np.arange(128)[None, :]
    mask = (kk <= qq).astype(bf)  # [128,128] fine-diagonal causal mask
    ident = np.eye(128, dtype=bf)

    wot = np.ascontiguousarray(wo.T).astype(bf)  # [D feat, D out]

    in_maps = []
    for c in range(N_CORES):
        b, g = divmod(c, 4)
        xt = np.ascontiguousarray(x[b, :S].T).astype(bf)
        wqt = np.ascontiguousarray(
            wq_p[HG * g : HG * (g + 1)].reshape(GD, D_MODEL).T
        ).astype(bf)
        wkt = np.ascontiguousarray(wk_p[g].T).astype(bf)
        wvt = np.ascontiguousarray(wv[HEAD_DIM * g : HEAD_DIM * (g + 1)].T).astype(bf)
        in_maps.append(
            {
                "xt": xt,
                "wqt": wqt,
                "wkt": wkt,
                "wvt": wvt,
                "wot": wot,
                "cos": cos,
                "sin": sin,
                "mask": mask,
                "ident": ident,
            }
        )
    return in_maps


def run(x, wq, wk, wv, wo, S=None, trace=False):
    from concourse.bass_utils import run_bass_kernel_spmd

    if S is None:
        S = x.shape[1]
    if S not in _BUILD_CACHE:
        _BUILD_CACHE[S] = build_kernel(S)
    nc = _BUILD_CACHE[S]
    in_maps = host_prep(x, wq, wk, wv, wo, S)
    res = run_bass_kernel_spmd(nc, in_maps, core_ids=list(range(N_CORES)), trace=trace)
    out = np.empty((B, S, D_MODEL), np.float32)
    nj = S // 512
    for c in range(N_CORES):
        b, g = divmod(c, 4)
        o = res.results[c]["out"]  # [nj*128, D]: row blk*128+r = global 512*blk+128*g+r
        for blk in range(nj):
            out[b, 512 * blk + 128 * g : 512 * blk + 128 * (g + 1), :] = o[
                128 * blk : 128 * (blk + 1)
            ]
    return out, res


def kernel(x, wq, wk, wv, wo):
    x = np.asarray(x, np.float32)
    wq = np.asarray(wq, np.float32)
    wk = np.asarray(wk, np.float32)
    wv = np.asarray(wv, np.float32)
    wo = np.asarray(wo, np.float32)
    out, _ = run(x, wq, wk, wv, wo)
    return out


# revision 43
# speedup vs baseline: 1.1828x; 1.0369x over previous
"""GroupedQueryAttention Trainium2 kernel (8 NeuronCores, SPMD).

Sharding: core c -> (batch b = c // 4, kv-group g = c % 4).
Each core computes q/k/v projections for its 4 query heads + 1 kv head,
partial-RoPE, causal attention; the per-group attention outputs o are
exchanged with a per-tile AllToAll (each core ends up with the full
2048 o-features for its 128-token slice of the tile), then every core
applies the full out-projection locally -- no post-projection reduction
is needed and only the small pre-projection o travels on the links.

Pipeline (single TileContext, per q-tile j): project(j) -> attend(j)
(AllToAll fires per head-pair as soon as its o rows are normalized) ->
out-project(j-1).  The list scheduler fills PE gaps during the
ACT-bound attention stretches with projection / out-projection matmuls.

All device matmuls run in bf16 (fp32 PSUM accumulation). The host
pre-transposes the operands so the contraction dim lands on SBUF
partitions everywhere with no on-device transposes:
  xt   = x[b].T                  [D, S]
  wqt  = perm(wq)[group].T       [D, 512]   (rows RoPE-deinterleaved)
  wkt  = perm(wk)[group].T       [D, 128]
  wvt  = wv[group].T             [D, 128]
  wot  = wo.T                    [D, D]     (full; o features are global)
The RoPE deinterleave permutation reorders each head's first 64 dims to
[evens, odds]; since q and k use the same permutation, q.k dot products
are unchanged and it never needs undoing.

Causal structure: for the diagonal 128-row k-chunks only the q columns
at-or-right-of the chunk are computed (scores, exp, p@V and the
denominator matmul are all narrowed); the 128x128 block on the fine
diagonal is masked with a single lower-triangular mask.
"""

import math
import sys

sys.path.insert(0, "/opt/trn_rl_repo")

import numpy as np  # noqa: E402

D_MODEL = 2048
N_HEADS = 16
N_KV = 4
HEAD_DIM = 128
ROPE = 64
THETA = 10000.0
B = 2
HG = N_HEADS // N_KV  # 4 query heads per kv group
GD = HG * HEAD_DIM  # 512 o-features per group
N_CORES = 8
GROUPS_A2A = [[0, 1, 2, 3, 4, 5, 6, 7]]
GROUPS_AG = [[0, 1, 2, 3], [4, 5, 6, 7]]

_BUILD_CACHE: dict = {}


def build_kernel(S: int):
    """Build the per-core Bass program for sequence length S (multiple of 512)."""
    import concourse.bass as bass
    import concourse.mybir as mybir
    import concourse.tile as tile
    from concourse import bacc

    assert S % 512 == 0
    P = 128
    QT = 512  # q tile width
    NJ = S // QT  # q tiles
    NO = D_MODEL // P  # contraction chunks for projections (16)
    NS = S // P  # seq chunks of 128
    N2 = D_MODEL // QT  # out-proj column blocks (4)
    bf16 = mybir.dt.bfloat16
    f32 = mybir.dt.float32
    scale = 1.0 / math.sqrt(HEAD_DIM)

    nc = bacc.Bacc(None, target_bir_lowering=False, debug=False, num_devices=N_CORES)

    xt_d = nc.declare_dram_parameter("xt", [D_MODEL, S], bf16, isOutput=False)
    wqt_d = nc.declare_dram_parameter("wqt", [D_MODEL, GD], bf16, isOutput=False)
    wkt_d = nc.declare_dram_parameter("wkt", [D_MODEL, HEAD_DIM], bf16, isOutput=False)
    wvt_d = nc.declare_dram_parameter("wvt", [D_MODEL, HEAD_DIM], bf16, isOutput=False)
    wot_d = nc.declare_dram_parameter("wot", [D_MODEL, D_MODEL], bf16, isOutput=False)
    cos_d = nc.declare_dram_parameter("cos", [ROPE, S], bf16, isOutput=False)
    sin_d = nc.declare_dram_parameter("sin", [ROPE, S], bf16, isOutput=False)
    msk_d = nc.declare_dram_parameter("mask", [P, P], bf16, isOutput=False)
    idn_d = nc.declare_dram_parameter("ident", [P, P], bf16, isOutput=False)
    out_d = nc.declare_dram_parameter("out", [S // 4, D_MODEL], f32, isOutput=True)

    with tile.TileContext(nc) as tc:
        with (
            tc.tile_pool(name="persist", bufs=1) as persist,
            tc.tile_pool(name="dram", bufs=1, space="DRAM") as dram,
        ):
            # ---- persistent SBUF state ----
            k_sb = persist.tile([P, S], bf16)  # kT (rope'd)
            v_sb = persist.tile([P, NS, HEAD_DIM], bf16)  # v natural per chunk
            cos_sb = persist.tile([ROPE, S], bf16)
            sin_sb = persist.tile([ROPE, S], bf16)
            msk_sb = persist.tile([P, P], bf16)
            idn_sb = persist.tile([P, P], bf16)
            ones_sb = persist.tile([P, P], bf16)
            wkt_sb = persist.tile([P, NO, HEAD_DIM], bf16)
            wvt_sb = persist.tile([P, NO, HEAD_DIM], bf16)
            wqt_sb = persist.tile([P, NO, GD], bf16)
            wot_sb = persist.tile([P, NO, D_MODEL], bf16)

            # 8-core AllToAll: 8 shards of [2 heads x 128 dv, 128 tok]; the
            # cross-batch shards carry unread garbage (mesh needs >4 ranks,
            # so the 4-core exchange rides an 8-core op; each core touches
            # only its batch's 1024-row half via a dynamic offset).
            a2a_in = [
                [
                    dram.tile([8 * 2 * P, P], bf16, name=f"a2ai{j}_{h2}")
                    for h2 in range(2)
                ]
                for j in range(NJ)
            ]
            a2a_out = [
                [
                    dram.tile([8 * 2 * P, P], bf16, name=f"a2ao{j}_{h2}")
                    for h2 in range(2)
                ]
                for j in range(NJ)
            ]
            # last tile: per-head 4-core AllGather (lower floor than the
            # 8-core mesh A2A; latency is all that matters at the tail)
            ag_in_l = [
                dram.tile([P, QT], bf16, name=f"agil{h}") for h in range(HG)
            ]
            ag_out_l = [
                dram.tile([4 * P, QT], bf16, name=f"agol{h}") for h in range(HG)
            ]
            warm_in = dram.tile([8, P], bf16, name="warm_in")
            warm_out = dram.tile([8, P], bf16, name="warm_out")
            b_half = nc.sync.partition_id() // 4  # 0 or 1: my batch group
            g_idx = nc.sync.partition_id() % 4  # my kv-group within the batch

            # Bulk loads are split between the two HWDGE engines (SP + ACT)
            # and chunked/ordered so the first projection matmuls start as
            # early as possible: the k-projection needs wkt chunk c and xt
            # chunk o in order, everything else can trail.
            nc.vector.memset(ones_sb[:], 1.0)
            wkt_r = wkt_d.rearrange("(o p) m -> p o m", p=P)
            wvt_r = wvt_d.rearrange("(o p) m -> p o m", p=P)
            wqt_r = wqt_d.rearrange("(o p) m -> p o m", p=P)
            wot_r = wot_d.rearrange("(o p) m -> p o m", p=P)
            xt_r = xt_d.rearrange("(o p) s -> p o s", p=P)

            # tiny dummy collective: pays the Mesh first-op setup cost during
            # the weight preload instead of on the first real exchange
            nc.gpsimd.collective_compute(
                "AllToAll",
                mybir.AluOpType.bypass,
                replica_groups=GROUPS_A2A,
                ins=[warm_in.opt()],
                outs=[warm_out.opt()],
            )

            with (
                tc.tile_pool(name="xt_pool", bufs=2) as xt_pool,
                tc.tile_pool(name="q_pool", bufs=2) as q_pool,
                tc.tile_pool(name="o_pool", bufs=2) as o_pool,
                tc.tile_pool(name="vt_pool", bufs=2) as vt_pool,
                tc.tile_pool(name="pt_pool", bufs=4) as pt_pool,
                tc.tile_pool(name="rcp_pool", bufs=2) as rcp_pool,
                tc.tile_pool(name="orecv_pool", bufs=4) as orecv_pool,
                tc.tile_pool(name="ostg_pool", bufs=8) as ostg_pool,
                tc.tile_pool(name="rtmp", bufs=2) as rtmp,
                tc.tile_pool(name="ps_sc", bufs=2, space="PSUM") as ps_sc,
                tc.tile_pool(name="ps_mm", bufs=2, space="PSUM") as ps_mm,
                tc.tile_pool(name="ps_po", bufs=2, space="PSUM") as ps_po,
                tc.tile_pool(name="ps_den", bufs=2, space="PSUM") as ps_den,
            ):

                def load_xt(j):
                    t = xt_pool.tile([P, NO, QT], bf16, tag="xt")
                    for o2 in range(NO // 2):
                        nc.scalar.dma_start(
                            t[:, 2 * o2 : 2 * o2 + 2, :],
                            xt_r[:, 2 * o2 : 2 * o2 + 2, bass.ts(j, QT)],
                        )
                    return t

                def rope(dst, sl, csl):
                    # rotate-half form on deinterleaved rows:
                    #   rows 0:32 = a (even dims), 32:64 = b (odd dims)
                    #   new[0:64] = old[0:64]*cos64 + swap(old[0:64])*sin64
                    # with cos64 = [cosT; cosT], sin64 = [-sinT; sinT].
                    xs = rtmp.tile([64, QT], bf16, tag="xs")
                    nc.vector.tensor_copy(xs[0:32, :], dst[32:64, sl])
                    nc.vector.tensor_copy(xs[32:64, :], dst[0:32, sl])
                    t = rtmp.tile([64, QT], bf16, tag="t")
                    u = rtmp.tile([64, QT], bf16, tag="u")
                    nc.vector.tensor_mul(t[:], xs[:], sin_sb[:, csl])
                    nc.vector.tensor_mul(u[:], dst[0:64, sl], cos_sb[:, csl])
                    nc.vector.tensor_add(dst[0:64, sl], u[:], t[:])

                def oproj(j, pieces):
                    # pieces: [(recv_tile, [global chunk ids])]
                    # out rows [128j, 128j+128) = tokens 512j+128g..  (host maps)
                    flat = [
                        (t, i, c)
                        for (t, cids) in pieces
                        for i, c in enumerate(cids)
                    ]
                    for n2 in range(N2):
                        ps = ps_mm.tile([P, QT], f32, tag="mm")
                        for k, (t, i, c) in enumerate(flat):
                            nc.tensor.matmul(
                                ps[:],
                                t[:, i, :],
                                wot_sb[:, c, bass.ts(n2, QT)],
                                start=(k == 0),
                                stop=(k == len(flat) - 1),
                            )
                        ostg = ostg_pool.tile([P, QT], f32, tag="ostg")
                        nc.vector.tensor_copy(ostg[:], ps[:])
                        nc.sync.dma_start(
                            out_d[bass.ts(j, P), bass.ts(n2, QT)], ostg[:]
                        )

                # ---- startup loads, interleaved across both HWDGE engines so
                # the k-projection's chunk-o inputs land in consumption order
                xts = [None] * NJ
                xts[0] = xt_pool.tile([P, NO, QT], bf16, name="xt0", tag="xt")
                for o in range(NO):
                    if o % 4 == 0:
                        nc.scalar.dma_start(
                            wkt_sb[:, o : o + 4, :], wkt_r[:, o : o + 4, :]
                        )
                    eng = nc.scalar if o % 2 == 0 else nc.sync
                    eng.dma_start(xts[0][:, o, :], xt_r[:, o, bass.ts(0, QT)])
                    if o == 1:
                        # small-but-critical constants: needed by rope (~20us),
                        # the v transpose and the diagonal mask
                        nc.scalar.dma_start(cos_sb[:], cos_d[:])
                        nc.sync.dma_start(sin_sb[:], sin_d[:])
                        nc.scalar.dma_start(idn_sb[:], idn_d[:])
                        nc.sync.dma_start(msk_sb[:], msk_d[:])
                for c in range(4):
                    nc.scalar.dma_start(
                        wvt_sb[:, 4 * c : 4 * c + 4, :],
                        wvt_r[:, 4 * c : 4 * c + 4, :],
                    )
                # wqt by column halves (heads 0-1 then 2-3) so the first
                # q-head projections can start before the whole load lands
                for c in range(4):
                    nc.scalar.dma_start(
                        wqt_sb[:, 4 * c : 4 * c + 4, 0 : 2 * P],
                        wqt_r[:, 4 * c : 4 * c + 4, 0 : 2 * P],
                    )
                    nc.sync.dma_start(
                        wqt_sb[:, 4 * c : 4 * c + 4, 2 * P : 4 * P],
                        wqt_r[:, 4 * c : 4 * c + 4, 2 * P : 4 * P],
                    )
                for o in range(NO):
                    nc.sync.dma_start(wot_sb[:, o, :], wot_r[:, o, :])
                all_pieces = [None] * NJ

                for j in range(NJ):
                    jsl = bass.ts(j, QT)
                    if j + 1 < NJ:
                        xts[j + 1] = load_xt(j + 1)
                    xt_sb = xts[j]

                    # ---- projections (+RoPE) for tile j ----
                    ps = ps_mm.tile([P, QT], f32, tag="mm")
                    for o in range(NO):
                        nc.tensor.matmul(
                            ps[:], wkt_sb[:, o, :], xt_sb[:, o, :],
                            start=(o == 0), stop=(o == NO - 1),
                        )
                    nc.vector.tensor_copy(k_sb[:, jsl], ps[:])
                    rope(k_sb, jsl, jsl)

                    ps = ps_mm.tile([P, QT], f32, tag="mm")
                    for o in range(NO):
                        nc.tensor.matmul(
                            ps[:], wvt_sb[:, o, :], xt_sb[:, o, :],
                            start=(o == 0), stop=(o == NO - 1),
                        )
                    vt_sb = vt_pool.tile([P, QT], bf16, tag="vt")
                    nc.vector.tensor_copy(vt_sb[:], ps[:])
                    for i in range(QT // P):
                        # PE transpose-mode: [dv, tok] -> [tok, dv], no DMA
                        tps = ps_mm.tile([P, P], bf16, tag="mm")
                        nc.tensor.transpose(
                            tps[:], vt_sb[:, bass.ts(i, P)], idn_sb[:]
                        )
                        nc.vector.tensor_copy(v_sb[:, 4 * j + i, :], tps[:])

                    q_sb = q_pool.tile([P, HG, QT], bf16, tag="q")
                    for h in range(HG):
                        ps = ps_mm.tile([P, QT], f32, tag="mm")
                        for o in range(NO):
                            nc.tensor.matmul(
                                ps[:], wqt_sb[:, o, bass.ts(h, P)], xt_sb[:, o, :],
                                start=(o == 0), stop=(o == NO - 1),
                            )
                        nc.vector.tensor_copy(q_sb[:, h, :], ps[:])
                        rope(q_sb[:, h, :], slice(0, QT), jsl)

                    # out-projection lags 2 tiles: the CC-init barrier makes
                    # early exchanges slow and very variable; committing the
                    # in-order PE stream to oproj only 2 tiles later keeps it
                    # from head-of-line blocking on a late AllToAll.
                    if j >= 2:
                        oproj(j - 2, all_pieces[j - 2])

                    # ---- attention for tile j ----
                    o_sb = o_pool.tile([P, HG, QT], bf16, tag="o")
                    for h in range(HG):
                        po = ps_po.tile([P, QT], f32, tag="po")
                        den = ps_den.tile([P, QT], f32, tag="den")
                        nk = 4 * (j + 1)
                        for c in range(nk):
                            r = c - 4 * j  # >=0 on the diagonal block
                            off = P * r if r >= 0 else 0
                            w = QT - off
                            sc = ps_sc.tile([P, QT], f32, tag="sc")
                            nc.tensor.matmul(
                                sc[:, 0:w],
                                k_sb[:, bass.ts(c, P)],
                                q_sb[:, h, off:QT],
                                start=True, stop=True,
                            )
                            pt = pt_pool.tile([P, QT], bf16, tag="pt")
                            nc.scalar.activation(
                                pt[:, 0:w], sc[:, 0:w],
                                mybir.ActivationFunctionType.Exp,
                                scale=scale,
                            )
                            if r >= 0:
                                nc.vector.tensor_mul(
                                    pt[:, 0:P], pt[:, 0:P], msk_sb[:]
                                )
                            nc.tensor.matmul(
                                po[:, off:QT], v_sb[:, c, :], pt[:, 0:w],
                                start=(c == 0), stop=(c == nk - 1),
                                skip_group_check=True,
                            )
                            nc.tensor.matmul(
                                den[:, off:QT], ones_sb[:], pt[:, 0:w],
                                start=(c == 0), stop=(c == nk - 1),
                                skip_group_check=True,
                            )
                        rcp = rcp_pool.tile([P, QT], f32, tag="rcp")
                        nc.vector.reciprocal_approx_fast(rcp[:], den[:])
                        nc.vector.tensor_mul(o_sb[:, h, :], po[:], rcp[:])

                        if j == NJ - 1:
                            # last tile: exchange each head as soon as it is
                            # ready, so the final op (and its oproj chunk)
                            # is as small and early as possible.
                            nc.sync.dma_start(ag_in_l[h][:], o_sb[:, h, :])
                            nc.gpsimd.collective_compute(
                                "AllGather",
                                mybir.AluOpType.bypass,
                                replica_groups=GROUPS_AG,
                                ins=[ag_in_l[h].opt()],
                                outs=[ag_out_l[h].opt()],
                            )
                        elif h % 2 == 1:
                            # AllToAll this head-pair's o: shard s carries my
                            # 2 heads x 128 dv for the 128 tokens core s owns.
                            half = h // 2
                            a2a_in_v = a2a_in[j][half].rearrange(
                                "(d hh p) t -> hh p d t", d=8, hh=2, p=P
                            )
                            for hh in range(2):
                                nc.sync.dma_start(
                                    a2a_in_v[hh, :, bass.ts(b_half, 4), :],
                                    o_sb[:, 2 * half + hh, :].rearrange(
                                        "p (s t) -> p s t", s=4
                                    ),
                                )
                            nc.gpsimd.collective_compute(
                                "AllToAll",
                                mybir.AluOpType.bypass,
                                replica_groups=GROUPS_A2A,
                                ins=[a2a_in[j][half].opt()],
                                outs=[a2a_out[j][half].opt()],
                            )

                    if j < NJ - 1:
                        prev_pieces = []
                        for half in range(2):
                            t = orecv_pool.tile([P, 8, P], bf16, tag="orecv")
                            nc.sync.dma_start(
                                t[:],
                                a2a_out[j][half].rearrange(
                                    "(x p) t -> p x t", p=P
                                )[:, bass.ts(b_half, 8), :],
                            )
                            cids = [
                                4 * (i // 2) + 2 * half + (i % 2) for i in range(8)
                            ]
                            prev_pieces.append((t, cids))
                    else:
                        prev_pieces = []
                        for h in range(HG):
                            t = orecv_pool.tile([P, 4, P], bf16, tag="orecvl")
                            nc.sync.dma_start(
                                t[:],
                                ag_out_l[h].rearrange("(s p) t -> p s t", p=P)[
                                    :, :, bass.ts(g_idx, P)
                                ],
                            )
                            prev_pieces.append((t, [4 * s + h for s in range(4)]))
                    all_pieces[j] = prev_pieces

                oproj(NJ - 2, all_pieces[NJ - 2])
                prev_pieces = all_pieces[NJ - 1]
                # last tile: accumulate heads 0-2 while head 3's exchange is
                # still in flight, then only the 4 head-3 chunks trail it
                stg = []
                for n2 in range(N2):
                    ps = ps_mm.tile([P, QT], f32, tag="mm")
                    flat = [
                        (t, i, c)
                        for (t, cids) in prev_pieces[:3]
                        for i, c in enumerate(cids)
                    ]
                    for k, (t, i, c) in enumerate(flat):
                        nc.tensor.matmul(
                            ps[:],
                            t[:, i, :],
                            wot_sb[:, c, bass.ts(n2, QT)],
                            start=(k == 0),
                            stop=(k == len(flat) - 1),
                        )
                    ostg = ostg_pool.tile([P, QT], f32, tag="ostg")
                    nc.vector.tensor_copy(ostg[:], ps[:])
                    stg.append(ostg)
                t3, cids3 = prev_pieces[3]
                for n2 in range(N2):
                    ps = ps_mm.tile([P, QT], f32, tag="mm")
                    for i, c in enumerate(cids3):
                        nc.tensor.matmul(
                            ps[:],
                            t3[:, i, :],
                            wot_sb[:, c, bass.ts(n2, QT)],
                            start=(i == 0),
                            stop=(i == len(cids3) - 1),
                        )
                    nc.vector.tensor_add(stg[n2][:], stg[n2][:], ps[:])
                    nc.sync.dma_start(
                        out_d[bass.ts(NJ - 1, P), bass.ts(n2, QT)], stg[n2][:]
                    )

    nc.compile()
    return nc


def host_prep(x, wq, wk, wv, wo, S):
    """Build the 8 per-core input maps (numpy, bf16)."""
    import ml_dtypes

    bf = ml_dtypes.bfloat16
    perm = np.concatenate(
        [np.arange(0, ROPE, 2), np.arange(1, ROPE, 2), np.arange(ROPE, HEAD_DIM)]
    )
    wq_p = wq.reshape(N_HEADS, HEAD_DIM, D_MODEL)[:, perm, :]
    wk_p = wk.reshape(N_KV, HEAD_DIM, D_MODEL)[:, perm, :]

    inv = THETA ** (-np.arange(0, ROPE, 2, dtype=np.float64) / ROPE)  # [32]
    t = np.arange(S, dtype=np.float64)
    ang = np.outer(inv, t)  # [32, S]
    cosT, sinT = np.cos(ang), np.sin(ang)
    cos = np.ascontiguousarray(np.concatenate([cosT, cosT], 0)).astype(bf)  # [64,S]
    sin = np.ascontiguousarray(np.concatenate([-sinT, sinT], 0)).astype(bf)  # [64,S]

    kk = np.arange(128)[:, None]
    qq = np.arange(128)[None, :]
    mask = (kk <= qq).astype(bf)  # [128,128] fine-diagonal causal mask
    ident = np.eye(128, dtype=bf)

    wot = np.ascontiguousarray(wo.T).astype(bf)  # [D feat, D out]

    in_maps = []
    for c in range(N_CORES):
        b, g = divmod(c, 4)
        xt = np.ascontiguousarray(x[b, :S].T).astype(bf)
        wqt = np.ascontiguousarray(
            wq_p[HG * g : HG * (g + 1)].reshape(GD, D_MODEL).T
        ).astype(bf)
        wkt = np.ascontiguousarray(wk_p[g].T).astype(bf)
        wvt = np.ascontiguousarray(wv[HEAD_DIM * g : HEAD_DIM * (g + 1)].T).astype(bf)
        in_maps.append(
            {
                "xt": xt,
                "wqt": wqt,
                "wkt": wkt,
                "wvt": wvt,
                "wot": wot,
                "cos": cos,
                "sin": sin,
                "mask": mask,
                "ident": ident,
            }
        )
    return in_maps


def run(x, wq, wk, wv, wo, S=None, trace=False):
    from concourse.bass_utils import run_bass_kernel_spmd

    if S is None:
        S = x.shape[1]
    if S not in _BUILD_CACHE:
        _BUILD_CACHE[S] = build_kernel(S)
    nc = _BUILD_CACHE[S]
    in_maps = host_prep(x, wq, wk, wv, wo, S)
    res = run_bass_kernel_spmd(nc, in_maps, core_ids=list(range(N_CORES)), trace=trace)
    out = np.empty((B, S, D_MODEL), np.float32)
    nj = S // 512
    for c in range(N_CORES):
        b, g = divmod(c, 4)
        o = res.results[c]["out"]  # [nj*128, D]: row blk*128+r = global 512*blk+128*g+r
        for blk in range(nj):
            out[b, 512 * blk + 128 * g : 512 * blk + 128 * (g + 1), :] = o[
                128 * blk : 128 * (blk + 1)
            ]
    return out, res


def kernel(x, wq, wk, wv, wo):
    x = np.asarray(x, np.float32)
    wq = np.asarray(wq, np.float32)
    wk = np.asarray(wk, np.float32)
    wv = np.asarray(wv, np.float32)
    wo = np.asarray(wo, np.float32)
    out, _ = run(x, wq, wk, wv, wo)
    return out
